# revision 14
# baseline (speedup 1.0000x reference)
"""DigitCaps dynamic-routing kernel for 8 Trainium2 NeuronCores.

Math (reference):
    u: [512, 1152, 8]  W: [1152, 10, 16, 8]
    u_hat[b,n,c,i] = sum_j W[n,c,i,j] u[b,n,j]        (never materialized)
    b=0; for 3 iters: c=softmax(b,axis=1); s=einsum('bnci,nc->bci',u_hat,c);
                      v=squash(s); b+=einsum('bnci,bci->nc',u_hat,v)

Strategy: data-parallel over batch (64 per core). u_hat is recomputed
implicitly inside two factored matmuls per routing iteration:
  s[b,(c,i)]     = u_flat[b,(j,n)] @ (c*W)[(j,n),(c,i)]    (contract 9216)
  M[(j,n),(c,i)] = u_flat^T @ v_flat                       (contract 64)
  agreement a[n,c] = sum_{j,i} W[(j,n),(c,i)] * M[(j,n),(c,i)]   (DVE)
b is shared across batch -> per-core partial agreements are AllGathered
(cheaper than AllReduce at this size) and summed locally each iteration.
The 3rd iteration's agreement is dead code (b never read again) -> skipped.
Iteration 1's uniform c=0.1 is folded in as cw = 0.1*W (tensor_scalar, 2x).

Layouts (J-outer, flat nj = j*1152 + n, chunks k=(j,k2) of 128 rows):
  ut[(j,n)%128, (j? no) ...]: uT tile [128, (j,k2,b)]   s-matmul lhsT slices
  uf  [b, (j,n)] as 8 tiles [64, 1152]                  M-matmul lhsT slices
  w   [128, (j,k2,c,i)] single tile                     agreement/cw source
  cw  [128, (j,k2,ci padded to MW)]                     s-matmul rhs
MM_MODE "f32r" uses the replicated-fp32 PE mode (1 cy/row at out-free>=256,
vs 4 cy/row for exact fp32); hardware numerics differ from sim - verified
end-to-end on HW against the fp32 reference before adoption.
"""

import sys

sys.path.insert(0, "/opt/trn_rl_repo")

import numpy as np

N_CORES = 8
B = 512
BL = B // N_CORES  # 64 batch per core
N = 1152
C = 10
DI = 16  # output capsule dim
DJ = 8  # input capsule dim
CI = C * DI  # 160
NB = 9  # n-blocks of 128
MM_MODE = "fp32"  # "fp32" | "f32r"
import os as _os
S_TILE = _os.environ.get("S_TILE", "1") == "1"  # PE col-tiling of s-matmul
M_TILE = _os.environ.get("M_TILE", "0") == "1"  # row-tiled agreement mms (crashes HW runtime; off)

_CACHE = {}


def _build(mm_mode, collectives=True):
    import concourse.bacc as bacc
    import concourse.mybir as mybir
    import concourse.tile as tile

    f32 = mybir.dt.float32
    f32r = mybir.dt.float32r

    nc = bacc.Bacc("TRN2", target_bir_lowering=False, debug=False,
                   num_devices=N_CORES)

    ut_d = [nc.dram_tensor(f"ut{j}", [128, NB * BL], f32, kind="ExternalInput")
            for j in range(DJ)]
    if M_TILE:
        uf_d = [nc.dram_tensor(f"ufp{jj}", [2 * BL, N], f32,
                               kind="ExternalInput") for jj in range(DJ // 2)]
    else:
        uf_d = [nc.dram_tensor(f"uf{j}", [BL, N], f32, kind="ExternalInput")
                for j in range(DJ)]
    w_d = [nc.dram_tensor(f"w{j}", [128, NB * CI], f32, kind="ExternalInput")
           for j in range(DJ)]
    v_d = nc.dram_tensor("v", [BL, CI], f32, kind="ExternalOutput")

    def mm_cast(ap):
        return ap.bitcast(f32r) if mm_mode == "f32r" else ap

    MW = 256 if mm_mode == "f32r" else CI  # matmul rhs/out free width

    with tile.TileContext(nc) as tc:
        with (
            tc.tile_pool(name="const", bufs=1) as cpool,
            tc.tile_pool(name="work", bufs=3) as wpool,
            tc.tile_pool(name="psum", bufs=2, space="PSUM") as psum_pool,
            tc.tile_pool(name="dram", bufs=1, space="DRAM") as dpool,
        ):
            # ---- tiles ----
            w_sb = cpool.tile([128, DJ * NB * CI], f32, tag="w_sb")
            cw = cpool.tile([128, DJ * NB * MW], f32, tag="cw")
            ut = cpool.tile([128, DJ * NB * BL], f32, tag="ut")
            for j in range(DJ):
                nc.sync.dma_start(
                    w_sb[:, j * NB * CI:(j + 1) * NB * CI], w_d[j].ap())
                nc.sync.dma_start(
                    ut[:, j * NB * BL:(j + 1) * NB * BL], ut_d[j].ap())
            uf = []
            if M_TILE:
                for jj in range(DJ // 2):
                    t = cpool.tile([2 * BL, N], f32, tag=f"ufp{jj}",
                                   name=f"ufp{jj}s")
                    nc.sync.dma_start(t[:], uf_d[jj].ap())
                    uf.append(t)
            else:
                for j in range(DJ):
                    t = cpool.tile([BL, N], f32, tag=f"uf{j}",
                                   name=f"uf{j}s")
                    nc.sync.dma_start(t[:], uf_d[j].ap())
                    uf.append(t)

            v_sb = cpool.tile([2 * BL, MW], f32, tag="v_sb")
            if MW > CI:
                nc.vector.memset(v_sb[:, CI:MW], 0.0)
                nc.vector.memset(cw[:], 0.0)  # pad cols must be 0 once

            b_acc = cpool.tile([128, NB * C], f32, tag="b_acc")
            c_sb = cpool.tile([128, NB * C], f32, tag="c_sb")
            a_loc = cpool.tile([128, NB * C], f32, tag="a_loc")
            ag_sb = cpool.tile([128, N_CORES * NB * C], f32, tag="ag_sb")

            def wv(j):  # w view [128, k2, c, i] for one j
                return w_sb[:, j * NB * CI:(j + 1) * NB * CI].rearrange(
                    "p (k c i) -> p k c i", c=C, i=DI)

            def cwv(j):  # cw view [128, k2, c, i] (MW-strided) for one j
                return cw[:, j * NB * MW:(j + 1) * NB * MW].rearrange(
                    "p (k x) -> p k x", x=MW)[:, :, :CI].rearrange(
                    "p k (c i) -> p k c i", i=DI)

            def compute_cw1():
                # iteration-1 c is uniform 0.1: cw = 0.1 * w (single-src, 2x)
                for j in range(DJ):
                    nc.vector.tensor_scalar_mul(cwv(j), wv(j), 0.1)

            def compute_cw():
                cb = c_sb[:].rearrange("p (k c) -> p k c", c=C).to_broadcast(
                    (128, NB, C, DI))
                for j in range(DJ):
                    nc.vector.tensor_tensor(cwv(j), wv(j), cb,
                                            op=mybir.AluOpType.mult)

            def softmax():
                b3 = b_acc[:].rearrange("p (k c) -> p k c", c=C)
                m = wpool.tile([128, NB], f32, tag="sm_m")
                nc.vector.reduce_max(m[:], b3, axis=mybir.AxisListType.X)
                d = wpool.tile([128, NB * C], f32, tag="sm_d")
                nc.vector.tensor_tensor(
                    d[:].rearrange("p (k c) -> p k c", c=C), b3,
                    m[:].to_broadcast((128, NB, C)),
                    op=mybir.AluOpType.subtract)
                e = wpool.tile([128, NB * C], f32, tag="sm_e")
                nc.scalar.activation(e[:], d[:],
                                     mybir.ActivationFunctionType.Exp)
                ssum = wpool.tile([128, NB], f32, tag="sm_s")
                nc.vector.reduce_sum(
                    ssum[:], e[:].rearrange("p (k c) -> p k c", c=C),
                    axis=mybir.AxisListType.X)
                rec = wpool.tile([128, NB], f32, tag="sm_r")
                nc.vector.reciprocal(rec[:], ssum[:])
                nc.vector.tensor_tensor(
                    c_sb[:].rearrange("p (k c) -> p k c", c=C),
                    e[:].rearrange("p (k c) -> p k c", c=C),
                    rec[:].to_broadcast((128, NB, C)), op=mybir.AluOpType.mult)

            def s_matmul(rhs=None, rw=None):
                # batch split into two 32-row halves on distinct PE
                # column-groups -> the two accumulation chains run
                # concurrently on the 128x128 array (col tiling).
                rhs = cw if rhs is None else rhs
                rw = MW if rw is None else rw
                HB = BL // 2
                ps_a = psum_pool.tile([BL, 8 * 256], f32, tag="ps",
                                      name="ps_sa")[:, :MW]
                ps_b = psum_pool.tile([BL, 8 * 256], f32, tag="ps",
                                      name="ps_sb")[:, :MW]
                nk = DJ * NB
                if S_TILE:
                    for k in range(nk):
                        nc.tensor.matmul(
                            ps_a[:HB, :],
                            mm_cast(ut[:, k * BL:k * BL + HB]),
                            mm_cast(rhs[:, k * rw:k * rw + rw]),
                            start=(k == 0), stop=(k == nk - 1),
                            tile_position=(0, 0))
                        nc.tensor.matmul(
                            ps_b[HB:BL, :],
                            mm_cast(ut[:, k * BL + HB:(k + 1) * BL]),
                            mm_cast(rhs[:, k * rw:k * rw + rw]),
                            start=(k == 0), stop=(k == nk - 1),
                            tile_position=(0, HB))
                else:
                    for k in range(nk):
                        nc.tensor.matmul(
                            ps_a[:BL, :],
                            mm_cast(ut[:, k * BL:(k + 1) * BL]),
                            mm_cast(rhs[:, k * rw:k * rw + rw]),
                            start=(k == 0), stop=(k == nk - 1))
                return (ps_a, ps_b)

            def squash(ps, alpha=1.0):
                # s_true = alpha*ps; v = g*ps with
                # g = alpha^2*sqrt(q)/(1+alpha^2*q), q = sum_i ps^2
                a2 = alpha * alpha
                ps_a, ps_b = ps
                if not S_TILE:
                    ps_b = ps_a
                HB = BL // 2
                sq = wpool.tile([BL, CI], f32, tag="sq")
                nc.scalar.square(sq[:HB, :], ps_a[:HB, :CI])
                nc.scalar.square(sq[HB:BL, :], ps_b[HB:BL, :CI])
                q = wpool.tile([BL, C], f32, tag="q")
                nc.vector.reduce_sum(
                    q[:], sq[:].rearrange("p (c i) -> p c i", i=DI),
                    axis=mybir.AxisListType.X)
                rt = wpool.tile([BL, C], f32, tag="rt")
                nc.scalar.sqrt(rt[:], q[:])
                den = wpool.tile([BL, C], f32, tag="den")
                nc.scalar.activation(den[:], q[:],
                                     mybir.ActivationFunctionType.Identity,
                                     bias=1.0, scale=a2)
                dr = wpool.tile([BL, C], f32, tag="dr")
                nc.vector.reciprocal(dr[:], den[:])
                g = wpool.tile([BL, C], f32, tag="g")
                nc.vector.tensor_tensor(g[:], rt[:], dr[:],
                                        op=mybir.AluOpType.mult)
                # v = (s*a2) * g  (g broadcast over i)
                nc.vector.scalar_tensor_tensor(
                    v_sb[:HB, :CI].rearrange("p (c i) -> p c i", i=DI),
                    ps_a[:HB, :CI].rearrange("p (c i) -> p c i", i=DI), a2,
                    g[:HB, :].to_broadcast((HB, C, DI)),
                    op0=mybir.AluOpType.mult, op1=mybir.AluOpType.mult)
                nc.vector.scalar_tensor_tensor(
                    v_sb[HB:BL, :CI].rearrange("p (c i) -> p c i", i=DI),
                    ps_b[HB:BL, :CI].rearrange("p (c i) -> p c i", i=DI), a2,
                    g[HB:BL, :].to_broadcast((HB, C, DI)),
                    op0=mybir.AluOpType.mult, op1=mybir.AluOpType.mult)
                if M_TILE:
                    # duplicate v into partitions 64..127 (row-tiled mm rhs)
                    nc.sync.dma_start(v_sb[BL:2 * BL, :CI], v_sb[:BL, :CI])

            def agreement():
                """a_loc[n%128, (k2,c)] = sum_{j,i} w * (uf^T @ v)."""
                for k2 in range(NB):
                    pm = psum_pool.tile([128, 8 * 256], f32, tag="ps",
                                        name="ps_m").rearrange(
                        "p (j x) -> p j x", x=256)
                    if M_TILE:
                        for jj in range(DJ // 2):
                            nc.tensor.matmul(
                                pm[:, 2 * jj, :MW],
                                mm_cast(uf[jj][:BL,
                                               k2 * 128:(k2 + 1) * 128]),
                                mm_cast(v_sb[:BL, :]),
                                start=True, stop=True, tile_position=(0, 0))
                            nc.tensor.matmul(
                                pm[:, 2 * jj + 1, :MW],
                                mm_cast(uf[jj][BL:2 * BL,
                                               k2 * 128:(k2 + 1) * 128]),
                                mm_cast(v_sb[BL:2 * BL, :]),
                                start=True, stop=True, tile_position=(BL, 0))
                    else:
                        for j in range(DJ):
                            nc.tensor.matmul(
                                pm[:, j, :MW],
                                mm_cast(uf[j][:, k2 * 128:(k2 + 1) * 128]),
                                mm_cast(v_sb[:BL, :]),
                                start=True, stop=True)
                    prod = wpool.tile([128, DJ * CI], f32, tag="prod")
                    nc.vector.tensor_tensor(
                        prod[:].rearrange("p (j x) -> p j x", x=CI),
                        w_sb[:].rearrange("p (j k x) -> p j k x",
                                          j=DJ, x=CI)[:, :, k2, :],
                        pm[:, :, :CI], op=mybir.AluOpType.mult)
                    # fused sum over (j, i): view [p, c, j, i], reduce XY
                    nc.vector.reduce_sum(
                        a_loc[:, k2 * C:(k2 + 1) * C],
                        prod[:].rearrange("p (j c i) -> p c j i",
                                          j=DJ, c=C, i=DI),
                        axis=mybir.AxisListType.XY)

            def allgather_and_update(first):
                it = int(first)
                cc_in = dpool.tile([128, NB * C], f32, name=f"cc_in{it}",
                                   tag=f"cc_in{it}")
                cc_out = dpool.tile([N_CORES, 128, NB * C], f32,
                                    addr_space="Shared", name=f"cc_out{it}",
                                    tag=f"cc_out{it}")
                nc.sync.dma_start(cc_in[:], a_loc[:])
                if collectives:
                    nc.gpsimd.collective_compute(
                        "AllGather", mybir.AluOpType.bypass,
                        replica_groups=[list(range(N_CORES))],
                        ins=[cc_in.opt()], outs=[cc_out.opt()])
                    nc.sync.dma_start(
                        ag_sb[:].rearrange("p (r x) -> p r x", r=N_CORES),
                        cc_out[:].transpose([1, 0, 2]))
                else:  # cost-sim stand-in for the gather
                    nc.sync.dma_start(ag_sb[:, :NB * C], cc_in[:])
                if first:
                    nc.vector.reduce_sum(
                        b_acc[:],
                        ag_sb[:].rearrange("p (r x) -> p x r", r=N_CORES),
                        axis=mybir.AxisListType.X)
                else:
                    asum = wpool.tile([128, NB * C], f32, tag="asum")
                    nc.vector.reduce_sum(
                        asum[:],
                        ag_sb[:].rearrange("p (r x) -> p x r", r=N_CORES),
                        axis=mybir.AxisListType.X)
                    nc.vector.tensor_tensor(b_acc[:], b_acc[:], asum[:],
                                            op=mybir.AluOpType.add)

            # ================= routing =================
            if mm_mode == "f32r":
                compute_cw1()
                ps = s_matmul()
                squash(ps, 1.0)
            else:  # fp32: no padding needed, use w directly with c=0.1 folded
                ps = s_matmul(rhs=w_sb, rw=CI)
                squash(ps, 0.1)
            agreement()
            allgather_and_update(first=True)

            softmax()
            compute_cw()
            ps = s_matmul()
            squash(ps)
            agreement()
            allgather_and_update(first=False)

            softmax()
            compute_cw()
            ps = s_matmul()
            squash(ps)
            nc.sync.dma_start(v_d.ap(), v_sb[:BL, :CI])

    nc.compile()
    return nc


def get_nc(mm_mode=MM_MODE, collectives=True):
    key = (mm_mode, collectives, S_TILE, M_TILE)
    if key not in _CACHE:
        _CACHE[key] = _build(mm_mode, collectives)
    return _CACHE[key]


def make_in_maps(u, W):
    """Host-side layout prep. u [512,1152,8] f32, W [1152,10,16,8] f32."""
    u = np.ascontiguousarray(u, dtype=np.float32)
    W = np.ascontiguousarray(W, dtype=np.float32)
    wj = W.transpose(3, 0, 1, 2).reshape(DJ, NB, 128, CI).transpose(0, 2, 1, 3)
    wj = np.ascontiguousarray(wj.reshape(DJ, 128, NB * CI))
    in_maps = []
    for core in range(N_CORES):
        ul = u[core * BL:(core + 1) * BL]  # [64, 1152, 8]
        utj = ul.transpose(2, 1, 0).reshape(DJ, NB, 128, BL).transpose(
            0, 2, 1, 3)
        utj = np.ascontiguousarray(utj.reshape(DJ, 128, NB * BL))
        ufj = np.ascontiguousarray(ul.transpose(2, 0, 1))  # [8, 64, 1152]
        ufp = ufj.reshape(DJ // 2, 2 * BL, N)  # pair (2jj, 2jj+1) stacked
        m = {}
        for j in range(DJ):
            m[f"ut{j}"] = utj[j]
            m[f"w{j}"] = wj[j]
        if M_TILE:
            for jj in range(DJ // 2):
                m[f"ufp{jj}"] = np.ascontiguousarray(ufp[jj])
        else:
            for j in range(DJ):
                m[f"uf{j}"] = ufj[j]
        in_maps.append(m)
    return in_maps


def kernel(u, W, _trace=False, _mm_mode=MM_MODE):
    from concourse import bass_utils

    nc = get_nc(_mm_mode)
    in_maps = make_in_maps(u, W)
    res = bass_utils.run_bass_kernel_spmd(
        nc, in_maps, core_ids=list(range(N_CORES)), trace=_trace)
    out = np.empty((B, C, DI), dtype=np.float32)
    for core in range(N_CORES):
        out[core * BL:(core + 1) * BL] = res.results[core]["v"].reshape(
            BL, C, DI)
    if _trace:
        kernel.last_results = res
    return out


# revision 18
# speedup vs baseline: 1.1134x; 1.1134x over previous
"""DigitCaps dynamic-routing kernel for 8 Trainium2 NeuronCores.

Math (reference):
    u: [512, 1152, 8]  W: [1152, 10, 16, 8]
    u_hat[b,n,c,i] = sum_j W[n,c,i,j] u[b,n,j]        (never materialized)
    b=0; for 3 iters: c=softmax(b,axis=1); s=einsum('bnci,nc->bci',u_hat,c);
                      v=squash(s); b+=einsum('bnci,bci->nc',u_hat,v)

Strategy: data-parallel over batch (64 per core). u_hat is recomputed
implicitly inside two factored matmuls per routing iteration:
  s[b,(c,i)]     = u_flat[b,(j,n)] @ (c*W)[(j,n),(c,i)]    (contract 9216)
  M[(j,n),(c,i)] = u_flat^T @ v_flat                       (contract 64)
  agreement a[n,c] = sum_{j,i} W[(j,n),(c,i)] * M[(j,n),(c,i)]   (DVE)
b is shared across batch -> per-core partial agreements are AllGathered
(cheaper than AllReduce at this size) and summed locally each iteration;
the gather is split into two k2-halves so the first AllGather hides under
the second half's compute. The 3rd iteration's agreement is dead code
(b never read again) -> skipped. Iteration 1's uniform c=0.1 uses W
directly with 0.1 folded into squash. Both matmul families run as pairs
of concurrent PE column-tile chains (tile_position col groups; row-group
offsets crash this runtime and are disabled). CW and the agreement
W*M multiplies are split DVE/GPSIMD (ACT evacuates PSUM for GPSIMD).

Layouts (J-outer, flat nj = j*1152 + n, chunks k=(j,k2) of 128 rows):
  ut[(j,n)%128, (j? no) ...]: uT tile [128, (j,k2,b)]   s-matmul lhsT slices
  uf  [b, (j,n)] as 8 tiles [64, 1152]                  M-matmul lhsT slices
  w   [128, (j,k2,c,i)] single tile                     agreement/cw source
  cw  [128, (j,k2,ci padded to MW)]                     s-matmul rhs
MM_MODE "f32r" uses the replicated-fp32 PE mode (1 cy/row at out-free>=256,
vs 4 cy/row for exact fp32); hardware numerics differ from sim - verified
end-to-end on HW against the fp32 reference before adoption.
"""

import sys

sys.path.insert(0, "/opt/trn_rl_repo")

import numpy as np

N_CORES = 8
B = 512
BL = B // N_CORES  # 64 batch per core
N = 1152
C = 10
DI = 16  # output capsule dim
DJ = 8  # input capsule dim
CI = C * DI  # 160
NB = 9  # n-blocks of 128
MM_MODE = "fp32"  # "fp32" | "f32r"
import os as _os
S_TILE = _os.environ.get("S_TILE", "1") == "1"  # PE col-tiling of s-matmul
M_TILE = _os.environ.get("M_TILE", "0") == "1"  # row-tiled agreement mms
G_SPLIT = _os.environ.get("G_SPLIT", "1") == "1"  # GPSIMD offload of DVE work
MC_TILE = _os.environ.get("MC_TILE", "1") == "1"  # col-tiled agreement mms

_CACHE = {}


def _build(mm_mode, collectives=True):
    import concourse.bacc as bacc
    import concourse.mybir as mybir
    import concourse.tile as tile

    f32 = mybir.dt.float32
    f32r = mybir.dt.float32r

    nc = bacc.Bacc("TRN2", target_bir_lowering=False, debug=False,
                   num_devices=N_CORES)

    ut_d = [nc.dram_tensor(f"ut{j}", [128, NB * BL], f32, kind="ExternalInput")
            for j in range(DJ)]
    if M_TILE:
        uf_d = [nc.dram_tensor(f"ufp{jj}", [2 * BL, N], f32,
                               kind="ExternalInput") for jj in range(DJ // 2)]
    else:
        uf_d = [nc.dram_tensor(f"uf{j}", [BL, N], f32, kind="ExternalInput")
                for j in range(DJ)]
    w_d = [nc.dram_tensor(f"w{j}", [128, NB * CI], f32, kind="ExternalInput")
           for j in range(DJ)]
    v_d = nc.dram_tensor("v", [BL, CI], f32, kind="ExternalOutput")
    dup_d = (nc.dram_tensor("dupm", [BL, 128], f32, kind="ExternalInput")
             if M_TILE else None)

    def mm_cast(ap):
        return ap.bitcast(f32r) if mm_mode == "f32r" else ap

    MW = 256 if mm_mode == "f32r" else CI  # matmul rhs/out free width

    with tile.TileContext(nc) as tc:
        with (
            tc.tile_pool(name="const", bufs=1) as cpool,
            tc.tile_pool(name="work", bufs=3) as wpool,
            tc.tile_pool(name="psum", bufs=2, space="PSUM") as psum_pool,
            tc.tile_pool(name="dram", bufs=1, space="DRAM") as dpool,
        ):
            # ---- tiles ----
            w_sb = cpool.tile([128, DJ * NB * CI], f32, tag="w_sb")
            cw = cpool.tile([128, DJ * NB * MW], f32, tag="cw")
            ut = cpool.tile([128, DJ * NB * BL], f32, tag="ut")
            for j in range(DJ):
                nc.sync.dma_start(
                    w_sb[:, j * NB * CI:(j + 1) * NB * CI], w_d[j].ap())
                nc.sync.dma_start(
                    ut[:, j * NB * BL:(j + 1) * NB * BL], ut_d[j].ap())
            uf = []
            if M_TILE:
                for jj in range(DJ // 2):
                    t = cpool.tile([2 * BL, N], f32, tag=f"ufp{jj}",
                                   name=f"ufp{jj}s")
                    nc.sync.dma_start(t[:], uf_d[jj].ap())
                    uf.append(t)
            else:
                for j in range(DJ):
                    t = cpool.tile([BL, N], f32, tag=f"uf{j}",
                                   name=f"uf{j}s")
                    nc.sync.dma_start(t[:], uf_d[j].ap())
                    uf.append(t)

            v_sb = cpool.tile([2 * BL, MW], f32, tag="v_sb")
            if M_TILE:
                dup_sb = cpool.tile([BL, 128], f32, tag="dup_sb")
                nc.sync.dma_start(dup_sb[:], dup_d.ap())
            if MW > CI:
                nc.vector.memset(v_sb[:, CI:MW], 0.0)
                nc.vector.memset(cw[:], 0.0)  # pad cols must be 0 once

            b_acc = cpool.tile([128, NB * C], f32, tag="b_acc")
            c_sb = cpool.tile([128, NB * C], f32, tag="c_sb")
            a_loc = cpool.tile([128, NB * C], f32, tag="a_loc")
            ag_sb = cpool.tile([128, N_CORES * NB * C], f32, tag="ag_sb")

            def wv(j):  # w view [128, k2, c, i] for one j
                return w_sb[:, j * NB * CI:(j + 1) * NB * CI].rearrange(
                    "p (k c i) -> p k c i", c=C, i=DI)

            def cwv(j):  # cw view [128, k2, c, i] (MW-strided) for one j
                return cw[:, j * NB * MW:(j + 1) * NB * MW].rearrange(
                    "p (k x) -> p k x", x=MW)[:, :, :CI].rearrange(
                    "p k (c i) -> p k c i", i=DI)

            JV = 5 if G_SPLIT else DJ  # j's on DVE; rest on GPSIMD

            def compute_cw1():
                # iteration-1 c is uniform 0.1: cw = 0.1 * w (single-src, 2x)
                for j in range(DJ):
                    eng = nc.vector if j < JV else nc.gpsimd
                    eng.tensor_scalar_mul(cwv(j), wv(j), 0.1)

            def compute_cw():
                cb = c_sb[:].rearrange("p (k c) -> p k c", c=C).to_broadcast(
                    (128, NB, C, DI))
                for j in range(DJ):
                    eng = nc.vector if j < JV else nc.gpsimd
                    eng.tensor_tensor(cwv(j), wv(j), cb,
                                      op=mybir.AluOpType.mult)

            def softmax():
                b3 = b_acc[:].rearrange("p (k c) -> p k c", c=C)
                m = wpool.tile([128, NB], f32, tag="sm_m")
                nc.vector.reduce_max(m[:], b3, axis=mybir.AxisListType.X)
                d = wpool.tile([128, NB * C], f32, tag="sm_d")
                nc.vector.tensor_tensor(
                    d[:].rearrange("p (k c) -> p k c", c=C), b3,
                    m[:].to_broadcast((128, NB, C)),
                    op=mybir.AluOpType.subtract)
                e = wpool.tile([128, NB * C], f32, tag="sm_e")
                nc.scalar.activation(e[:], d[:],
                                     mybir.ActivationFunctionType.Exp)
                ssum = wpool.tile([128, NB], f32, tag="sm_s")
                nc.vector.reduce_sum(
                    ssum[:], e[:].rearrange("p (k c) -> p k c", c=C),
                    axis=mybir.AxisListType.X)
                rec = wpool.tile([128, NB], f32, tag="sm_r")
                nc.vector.reciprocal(rec[:], ssum[:])
                nc.vector.tensor_tensor(
                    c_sb[:].rearrange("p (k c) -> p k c", c=C),
                    e[:].rearrange("p (k c) -> p k c", c=C),
                    rec[:].to_broadcast((128, NB, C)), op=mybir.AluOpType.mult)

            def s_matmul(rhs=None, rw=None):
                # batch split into two 32-row halves on distinct PE
                # column-groups -> the two accumulation chains run
                # concurrently on the 128x128 array (col tiling).
                rhs = cw if rhs is None else rhs
                rw = MW if rw is None else rw
                HB = BL // 2
                ps_a = psum_pool.tile([BL, 8 * 256], f32, tag="ps",
                                      name="ps_sa")[:, :MW]
                ps_b = psum_pool.tile([BL, 8 * 256], f32, tag="ps",
                                      name="ps_sb")[:, :MW]
                nk = DJ * NB
                if S_TILE:
                    for k in range(nk):
                        nc.tensor.matmul(
                            ps_a[:HB, :],
                            mm_cast(ut[:, k * BL:k * BL + HB]),
                            mm_cast(rhs[:, k * rw:k * rw + rw]),
                            start=(k == 0), stop=(k == nk - 1),
                            tile_position=(0, 0))
                        nc.tensor.matmul(
                            ps_b[HB:BL, :],
                            mm_cast(ut[:, k * BL + HB:(k + 1) * BL]),
                            mm_cast(rhs[:, k * rw:k * rw + rw]),
                            start=(k == 0), stop=(k == nk - 1),
                            tile_position=(0, HB))
                else:
                    for k in range(nk):
                        nc.tensor.matmul(
                            ps_a[:BL, :],
                            mm_cast(ut[:, k * BL:(k + 1) * BL]),
                            mm_cast(rhs[:, k * rw:k * rw + rw]),
                            start=(k == 0), stop=(k == nk - 1))
                return (ps_a, ps_b)

            def squash(ps, alpha=1.0):
                # s_true = alpha*ps; v = g*ps with
                # g = alpha^2*sqrt(q)/(1+alpha^2*q), q = sum_i ps^2
                a2 = alpha * alpha
                ps_a, ps_b = ps
                if not S_TILE:
                    ps_b = ps_a
                HB = BL // 2
                sq = wpool.tile([BL, CI], f32, tag="sq")
                nc.scalar.square(sq[:HB, :], ps_a[:HB, :CI])
                nc.scalar.square(sq[HB:BL, :], ps_b[HB:BL, :CI])
                q = wpool.tile([BL, C], f32, tag="q")
                nc.vector.reduce_sum(
                    q[:], sq[:].rearrange("p (c i) -> p c i", i=DI),
                    axis=mybir.AxisListType.X)
                rt = wpool.tile([BL, C], f32, tag="rt")
                nc.scalar.sqrt(rt[:], q[:])
                den = wpool.tile([BL, C], f32, tag="den")
                nc.scalar.activation(den[:], q[:],
                                     mybir.ActivationFunctionType.Identity,
                                     bias=1.0, scale=a2)
                dr = wpool.tile([BL, C], f32, tag="dr")
                nc.vector.reciprocal(dr[:], den[:])
                g = wpool.tile([BL, C], f32, tag="g")
                nc.vector.tensor_tensor(g[:], rt[:], dr[:],
                                        op=mybir.AluOpType.mult)
                # v = (s*a2) * g  (g broadcast over i)
                nc.vector.scalar_tensor_tensor(
                    v_sb[:HB, :CI].rearrange("p (c i) -> p c i", i=DI),
                    ps_a[:HB, :CI].rearrange("p (c i) -> p c i", i=DI), a2,
                    g[:HB, :].to_broadcast((HB, C, DI)),
                    op0=mybir.AluOpType.mult, op1=mybir.AluOpType.mult)
                nc.vector.scalar_tensor_tensor(
                    v_sb[HB:BL, :CI].rearrange("p (c i) -> p c i", i=DI),
                    ps_b[HB:BL, :CI].rearrange("p (c i) -> p c i", i=DI), a2,
                    g[HB:BL, :].to_broadcast((HB, C, DI)),
                    op0=mybir.AluOpType.mult, op1=mybir.AluOpType.mult)
                if M_TILE:
                    # duplicate v into partitions 64..127 via PE + ACT
                    # (cross-partition move; DMA SBUF->SBUF crashed HW)
                    ps_vd = psum_pool.tile([128, 8 * 256], f32, tag="ps",
                                           name="ps_vd")[:, :CI]
                    nc.tensor.matmul(ps_vd[:], dup_sb[:], v_sb[:BL, :CI],
                                     start=True, stop=True)
                    nc.scalar.copy(v_sb[BL:2 * BL, :CI],
                                   ps_vd[BL:2 * BL, :])

            def agreement(k2_lo, k2_hi):
                """a_loc[n%128, (k2,c)] = sum_{j,i} w * (uf^T @ v)."""
                for k2 in range(k2_lo, k2_hi):
                    pm = psum_pool.tile([128, 8 * 256], f32, tag="ps",
                                        name="ps_m").rearrange(
                        "p (j x) -> p j x", x=256)
                    if M_TILE:
                        for jj in range(DJ // 2):
                            nc.tensor.matmul(
                                pm[:, 2 * jj, :MW],
                                mm_cast(uf[jj][:BL,
                                               k2 * 128:(k2 + 1) * 128]),
                                mm_cast(v_sb[:BL, :]),
                                start=True, stop=True, tile_position=(0, 0))
                            nc.tensor.matmul(
                                pm[:, 2 * jj + 1, :MW],
                                mm_cast(uf[jj][BL:2 * BL,
                                               k2 * 128:(k2 + 1) * 128]),
                                mm_cast(v_sb[BL:2 * BL, :]),
                                start=True, stop=True, tile_position=(BL, 0))
                    elif MC_TILE:
                        # split the 128-wide n-chunk across PE column
                        # groups: two concurrent 64-col matmuls
                        for j in range(DJ):
                            nc.tensor.matmul(
                                pm[:BL, j, :MW],
                                mm_cast(uf[j][:, k2 * 128:k2 * 128 + BL]),
                                mm_cast(v_sb[:BL, :]),
                                start=True, stop=True, tile_position=(0, 0))
                            nc.tensor.matmul(
                                pm[BL:128, j, :MW],
                                mm_cast(uf[j][:, k2 * 128 + BL:
                                              (k2 + 1) * 128]),
                                mm_cast(v_sb[:BL, :]),
                                start=True, stop=True, tile_position=(0, BL))
                    else:
                        for j in range(DJ):
                            nc.tensor.matmul(
                                pm[:, j, :MW],
                                mm_cast(uf[j][:, k2 * 128:(k2 + 1) * 128]),
                                mm_cast(v_sb[:BL, :]),
                                start=True, stop=True)
                    prod = wpool.tile([128, DJ * CI], f32, tag="prod")
                    wslice = w_sb[:].rearrange("p (j k x) -> p j k x",
                                               j=DJ, x=CI)[:, :, k2, :]
                    if G_SPLIT and k2 % 2 == 1:
                        pmev = wpool.tile([128, DJ * CI], f32, tag="pmev")
                        nc.scalar.copy(
                            pmev[:].rearrange("p (j x) -> p j x", x=CI),
                            pm[:, :, :CI])
                        nc.gpsimd.tensor_tensor(
                            prod[:].rearrange("p (j x) -> p j x", x=CI),
                            wslice,
                            pmev[:].rearrange("p (j x) -> p j x", x=CI),
                            op=mybir.AluOpType.mult)
                    else:
                        nc.vector.tensor_tensor(
                            prod[:].rearrange("p (j x) -> p j x", x=CI),
                            wslice,
                            pm[:, :, :CI], op=mybir.AluOpType.mult)
                    # fused sum over (j, i): view [p, c, j, i], reduce XY
                    nc.vector.reduce_sum(
                        a_loc[:, k2 * C:(k2 + 1) * C],
                        prod[:].rearrange("p (j c i) -> p c j i",
                                          j=DJ, c=C, i=DI),
                        axis=mybir.AxisListType.XY)

            def allgather(it, half, k2_lo, k2_hi):
                """AllGather a_loc[:, 10*k2_lo:10*k2_hi] into ag_sb cols."""
                w0, w1 = k2_lo * C, k2_hi * C
                nw = w1 - w0
                cc_in = dpool.tile([128, nw], f32, name=f"cc_in{it}{half}",
                                   tag=f"cc_in{it}{half}")
                cc_out = dpool.tile([N_CORES, 128, nw], f32,
                                    addr_space="Shared",
                                    name=f"cc_out{it}{half}",
                                    tag=f"cc_out{it}{half}")
                nc.sync.dma_start(cc_in[:], a_loc[:, w0:w1])
                if collectives:
                    nc.gpsimd.collective_compute(
                        "AllGather", mybir.AluOpType.bypass,
                        replica_groups=[list(range(N_CORES))],
                        ins=[cc_in.opt()], outs=[cc_out.opt()])
                    nc.sync.dma_start(
                        ag_sb[:, w0 * N_CORES:w1 * N_CORES].rearrange(
                            "p (r x) -> p r x", r=N_CORES),
                        cc_out[:].transpose([1, 0, 2]))
                else:  # cost-sim stand-in for the gather
                    nc.sync.dma_start(ag_sb[:, w0 * N_CORES:w0 * N_CORES + nw],
                                      cc_in[:])

            def update_b(it, half, k2_lo, k2_hi, first):
                w0, w1 = k2_lo * C, k2_hi * C
                agv = ag_sb[:, w0 * N_CORES:w1 * N_CORES].rearrange(
                    "p (r x) -> p x r", r=N_CORES)
                if first:
                    nc.vector.reduce_sum(b_acc[:, w0:w1], agv,
                                         axis=mybir.AxisListType.X)
                else:
                    asum = wpool.tile([128, NB * C], f32, tag="asum")
                    nc.vector.reduce_sum(asum[:, w0:w1], agv,
                                         axis=mybir.AxisListType.X)
                    nc.vector.tensor_tensor(
                        b_acc[:, w0:w1], b_acc[:, w0:w1], asum[:, w0:w1],
                        op=mybir.AluOpType.add)

            KSPLIT = 5

            def agree_and_gather(it, first):
                agreement(0, KSPLIT)
                allgather(it, 0, 0, KSPLIT)  # fires under 2nd-half compute
                agreement(KSPLIT, NB)
                allgather(it, 1, KSPLIT, NB)
                update_b(it, 0, 0, KSPLIT, first)
                update_b(it, 1, KSPLIT, NB, first)

            # ================= routing =================
            # iter 1: uniform c -> use w directly, fold 0.1 into squash
            ps = s_matmul(rhs=w_sb, rw=CI)
            squash(ps, 0.1)
            agree_and_gather(0, first=True)

            softmax()
            compute_cw()
            ps = s_matmul()
            squash(ps)
            agree_and_gather(1, first=False)

            softmax()
            compute_cw()
            ps = s_matmul()
            squash(ps)
            nc.sync.dma_start(v_d.ap(), v_sb[:BL, :CI])

    nc.compile()
    return nc


def get_nc(mm_mode=MM_MODE, collectives=True):
    key = (mm_mode, collectives, S_TILE, M_TILE, G_SPLIT, MC_TILE)
    if key not in _CACHE:
        _CACHE[key] = _build(mm_mode, collectives)
    return _CACHE[key]


def make_in_maps(u, W):
    """Host-side layout prep. u [512,1152,8] f32, W [1152,10,16,8] f32."""
    u = np.ascontiguousarray(u, dtype=np.float32)
    W = np.ascontiguousarray(W, dtype=np.float32)
    wj = W.transpose(3, 0, 1, 2).reshape(DJ, NB, 128, CI).transpose(0, 2, 1, 3)
    wj = np.ascontiguousarray(wj.reshape(DJ, 128, NB * CI))
    in_maps = []
    for core in range(N_CORES):
        ul = u[core * BL:(core + 1) * BL]  # [64, 1152, 8]
        utj = ul.transpose(2, 1, 0).reshape(DJ, NB, 128, BL).transpose(
            0, 2, 1, 3)
        utj = np.ascontiguousarray(utj.reshape(DJ, 128, NB * BL))
        ufj = np.ascontiguousarray(ul.transpose(2, 0, 1))  # [8, 64, 1152]
        ufp = ufj.reshape(DJ // 2, 2 * BL, N)  # pair (2jj, 2jj+1) stacked
        m = {}
        for j in range(DJ):
            m[f"ut{j}"] = utj[j]
            m[f"w{j}"] = wj[j]
        if M_TILE:
            for jj in range(DJ // 2):
                m[f"ufp{jj}"] = np.ascontiguousarray(ufp[jj])
            m["dupm"] = np.ascontiguousarray(
                np.tile(np.eye(BL, dtype=np.float32), (1, 2)))
        else:
            for j in range(DJ):
                m[f"uf{j}"] = ufj[j]
        in_maps.append(m)
    return in_maps


def kernel(u, W, _trace=False, _mm_mode=MM_MODE):
    from concourse import bass_utils

    nc = get_nc(_mm_mode)
    in_maps = make_in_maps(u, W)
    res = bass_utils.run_bass_kernel_spmd(
        nc, in_maps, core_ids=list(range(N_CORES)), trace=_trace)
    out = np.empty((B, C, DI), dtype=np.float32)
    for core in range(N_CORES):
        out[core * BL:(core + 1) * BL] = res.results[core]["v"].reshape(
            BL, C, DI)
    if _trace:
        kernel.last_results = res
    return out


# revision 21
# speedup vs baseline: 1.1399x; 1.0238x over previous
"""DigitCaps dynamic-routing kernel for 8 Trainium2 NeuronCores.

Math (reference):
    u: [512, 1152, 8]  W: [1152, 10, 16, 8]
    u_hat[b,n,c,i] = sum_j W[n,c,i,j] u[b,n,j]        (never materialized)
    b=0; for 3 iters: c=softmax(b,axis=1); s=einsum('bnci,nc->bci',u_hat,c);
                      v=squash(s); b+=einsum('bnci,bci->nc',u_hat,v)

Strategy: data-parallel over batch (64 per core). u_hat is recomputed
implicitly inside two factored matmuls per routing iteration:
  s[b,(c,i)]     = u_flat[b,(j,n)] @ (c*W)[(j,n),(c,i)]    (contract 9216)
  M[(j,n),(c,i)] = u_flat^T @ v_flat                       (contract 64)
  agreement a[n,c] = sum_{j,i} W[(j,n),(c,i)] * M[(j,n),(c,i)]   (DVE)
b is shared across batch -> per-core partial agreements are AllGathered
(cheaper than AllReduce at this size) and summed locally each iteration;
the gather is split into two k2-halves so the first AllGather hides under
the second half's compute. The 3rd iteration's agreement is dead code
(b never read again) -> skipped. Iteration 1's uniform c=0.1 uses W
directly with 0.1 folded into squash. Both matmul families run as pairs
of concurrent PE column-tile chains (tile_position col groups; row-group
offsets crash this runtime and are disabled). CW and the agreement
W*M multiplies are split DVE/GPSIMD (ACT evacuates PSUM for GPSIMD).

Layouts (J-outer, flat nj = j*1152 + n, chunks k=(j,k2) of 128 rows):
  ut[(j,n)%128, (j? no) ...]: uT tile [128, (j,k2,b)]   s-matmul lhsT slices
  uf  [b, (j,n)] as 8 tiles [64, 1152]                  M-matmul lhsT slices
  w   [128, (j,k2,c,i)] single tile                     agreement/cw source
  cw  [128, (j,k2,ci padded to MW)]                     s-matmul rhs
MM_MODE "f32r" uses the replicated-fp32 PE mode (1 cy/row at out-free>=256,
vs 4 cy/row for exact fp32); hardware numerics differ from sim - verified
end-to-end on HW against the fp32 reference before adoption.
"""

import sys

sys.path.insert(0, "/opt/trn_rl_repo")

import numpy as np

N_CORES = 8
B = 512
BL = B // N_CORES  # 64 batch per core
N = 1152
C = 10
DI = 16  # output capsule dim
DJ = 8  # input capsule dim
CI = C * DI  # 160
NB = 9  # n-blocks of 128
MM_MODE = "fp32"  # "fp32" | "f32r"
import os as _os
S_TILE = _os.environ.get("S_TILE", "1") == "1"  # PE col-tiling of s-matmul
M_TILE = _os.environ.get("M_TILE", "0") == "1"  # row-tiled agreement mms
G_SPLIT = _os.environ.get("G_SPLIT", "1") == "1"  # GPSIMD offload of DVE work
MC_TILE = _os.environ.get("MC_TILE", "1") == "1"  # col-tiled agreement mms

_CACHE = {}


def _build(mm_mode, collectives=True):
    import concourse.bacc as bacc
    import concourse.mybir as mybir
    import concourse.tile as tile

    f32 = mybir.dt.float32
    f32r = mybir.dt.float32r

    nc = bacc.Bacc("TRN2", target_bir_lowering=False, debug=False,
                   num_devices=N_CORES)

    ut_d = [nc.dram_tensor(f"ut{j}", [128, NB * BL], f32, kind="ExternalInput")
            for j in range(DJ)]
    if M_TILE:
        uf_d = [nc.dram_tensor(f"ufp{jj}", [2 * BL, N], f32,
                               kind="ExternalInput") for jj in range(DJ // 2)]
    else:
        uf_d = [nc.dram_tensor(f"uf{j}", [BL, N], f32, kind="ExternalInput")
                for j in range(DJ)]
    w_d = [nc.dram_tensor(f"w{j}", [128, NB * CI], f32, kind="ExternalInput")
           for j in range(DJ)]
    v_d = nc.dram_tensor("v", [BL, CI], f32, kind="ExternalOutput")
    dup_d = (nc.dram_tensor("dupm", [BL, 128], f32, kind="ExternalInput")
             if M_TILE else None)

    def mm_cast(ap):
        return ap.bitcast(f32r) if mm_mode == "f32r" else ap

    MW = 256 if mm_mode == "f32r" else CI  # matmul rhs/out free width

    with tile.TileContext(nc) as tc:
        with (
            tc.tile_pool(name="const", bufs=1) as cpool,
            tc.tile_pool(name="work", bufs=3) as wpool,
            tc.tile_pool(name="psum", bufs=2, space="PSUM") as psum_pool,
            tc.tile_pool(name="dram", bufs=1, space="DRAM") as dpool,
        ):
            # ---- tiles ----
            w_sb = cpool.tile([128, DJ * NB * CI], f32, tag="w_sb")
            cw = cpool.tile([128, DJ * NB * MW], f32, tag="cw")
            ut = cpool.tile([128, DJ * NB * BL], f32, tag="ut")
            for j in range(DJ):
                nc.sync.dma_start(
                    w_sb[:, j * NB * CI:(j + 1) * NB * CI], w_d[j].ap())
                nc.sync.dma_start(
                    ut[:, j * NB * BL:(j + 1) * NB * BL], ut_d[j].ap())
            uf = []
            if M_TILE:
                for jj in range(DJ // 2):
                    t = cpool.tile([2 * BL, N], f32, tag=f"ufp{jj}",
                                   name=f"ufp{jj}s")
                    nc.sync.dma_start(t[:], uf_d[jj].ap())
                    uf.append(t)
            else:
                for j in range(DJ):
                    t = cpool.tile([BL, N], f32, tag=f"uf{j}",
                                   name=f"uf{j}s")
                    nc.sync.dma_start(t[:], uf_d[j].ap())
                    uf.append(t)

            v_sb = cpool.tile([2 * BL, MW], f32, tag="v_sb")
            if M_TILE:
                dup_sb = cpool.tile([BL, 128], f32, tag="dup_sb")
                nc.sync.dma_start(dup_sb[:], dup_d.ap())
            if MW > CI:
                nc.vector.memset(v_sb[:, CI:MW], 0.0)
                nc.vector.memset(cw[:], 0.0)  # pad cols must be 0 once

            b_acc = cpool.tile([128, NB * C], f32, tag="b_acc")
            c_sb = cpool.tile([128, NB * C], f32, tag="c_sb")
            a_loc = cpool.tile([128, NB * C], f32, tag="a_loc")
            ag_sb = cpool.tile([128, N_CORES * NB * C], f32, tag="ag_sb")

            def wv(j):  # w view [128, k2, c, i] for one j
                return w_sb[:, j * NB * CI:(j + 1) * NB * CI].rearrange(
                    "p (k c i) -> p k c i", c=C, i=DI)

            def cwv(j):  # cw view [128, k2, c, i] (MW-strided) for one j
                return cw[:, j * NB * MW:(j + 1) * NB * MW].rearrange(
                    "p (k x) -> p k x", x=MW)[:, :, :CI].rearrange(
                    "p k (c i) -> p k c i", i=DI)

            JV = 5 if G_SPLIT else DJ  # j's on DVE; rest on GPSIMD

            def compute_cw1():
                # iteration-1 c is uniform 0.1: cw = 0.1 * w (single-src, 2x)
                for j in range(DJ):
                    eng = nc.vector if j < JV else nc.gpsimd
                    eng.tensor_scalar_mul(cwv(j), wv(j), 0.1)

            def compute_cw(k2_lo=0, k2_hi=NB):
                nk = k2_hi - k2_lo
                cb = c_sb[:, k2_lo * C:k2_hi * C].rearrange(
                    "p (k c) -> p k c", c=C).to_broadcast((128, nk, C, DI))
                for j in range(DJ):
                    eng = nc.vector if j < JV else nc.gpsimd
                    eng.tensor_tensor(cwv(j)[:, k2_lo:k2_hi, :, :],
                                      wv(j)[:, k2_lo:k2_hi, :, :], cb,
                                      op=mybir.AluOpType.mult)

            def softmax(k2_lo=0, k2_hi=NB):
                # per-n row softmax: columns [k2_lo*C, k2_hi*C) are
                # independent of the rest -> can run per AllGather half
                nk = k2_hi - k2_lo
                w0, w1 = k2_lo * C, k2_hi * C
                b3 = b_acc[:, w0:w1].rearrange("p (k c) -> p k c", c=C)
                m = wpool.tile([128, NB], f32, tag="sm_m", name="sm_m")[:, :nk]
                nc.vector.reduce_max(m, b3, axis=mybir.AxisListType.X)
                d = wpool.tile([128, NB * C], f32, tag="sm_d", name="sm_d")[:, :nk * C]
                nc.vector.tensor_tensor(
                    d.rearrange("p (k c) -> p k c", c=C), b3,
                    m.to_broadcast((128, nk, C)),
                    op=mybir.AluOpType.subtract)
                e = wpool.tile([128, NB * C], f32, tag="sm_e", name="sm_e")[:, :nk * C]
                nc.scalar.activation(e, d,
                                     mybir.ActivationFunctionType.Exp)
                ssum = wpool.tile([128, NB], f32, tag="sm_s", name="sm_s")[:, :nk]
                nc.vector.reduce_sum(
                    ssum, e.rearrange("p (k c) -> p k c", c=C),
                    axis=mybir.AxisListType.X)
                rec = wpool.tile([128, NB], f32, tag="sm_r", name="sm_r")[:, :nk]
                nc.vector.reciprocal(rec, ssum)
                nc.vector.tensor_tensor(
                    c_sb[:, w0:w1].rearrange("p (k c) -> p k c", c=C),
                    e.rearrange("p (k c) -> p k c", c=C),
                    rec.to_broadcast((128, nk, C)), op=mybir.AluOpType.mult)

            def s_matmul(rhs=None, rw=None, k2_outer=True):
                # batch split into two 32-row halves on distinct PE
                # column-groups -> the two accumulation chains run
                # concurrently on the 128x128 array (col tiling).
                rhs = cw if rhs is None else rhs
                rw = MW if rw is None else rw
                HB = BL // 2
                ps_a = psum_pool.tile([BL, 8 * 256], f32, tag="ps",
                                      name="ps_sa")[:, :MW]
                ps_b = psum_pool.tile([BL, 8 * 256], f32, tag="ps",
                                      name="ps_sb")[:, :MW]
                nk = DJ * NB
                if k2_outer:  # half-A chunks first (AG-B still in flight)
                    order = [j * NB + k2 for k2 in range(NB)
                             for j in range(DJ)]
                else:  # j-outer: follows the per-j ut/w DMA arrival order
                    order = list(range(nk))
                if S_TILE:
                    for t, k in enumerate(order):
                        nc.tensor.matmul(
                            ps_a[:HB, :],
                            mm_cast(ut[:, k * BL:k * BL + HB]),
                            mm_cast(rhs[:, k * rw:k * rw + rw]),
                            start=(t == 0), stop=(t == nk - 1),
                            tile_position=(0, 0))
                        nc.tensor.matmul(
                            ps_b[HB:BL, :],
                            mm_cast(ut[:, k * BL + HB:(k + 1) * BL]),
                            mm_cast(rhs[:, k * rw:k * rw + rw]),
                            start=(t == 0), stop=(t == nk - 1),
                            tile_position=(0, HB))
                else:
                    for t, k in enumerate(order):
                        nc.tensor.matmul(
                            ps_a[:BL, :],
                            mm_cast(ut[:, k * BL:(k + 1) * BL]),
                            mm_cast(rhs[:, k * rw:k * rw + rw]),
                            start=(t == 0), stop=(t == nk - 1))
                return (ps_a, ps_b)

            def squash(ps, alpha=1.0):
                # s_true = alpha*ps; v = g*ps with
                # g = alpha^2*sqrt(q)/(1+alpha^2*q), q = sum_i ps^2
                a2 = alpha * alpha
                ps_a, ps_b = ps
                if not S_TILE:
                    ps_b = ps_a
                HB = BL // 2
                sq = wpool.tile([BL, CI], f32, tag="sq")
                nc.scalar.square(sq[:HB, :], ps_a[:HB, :CI])
                nc.scalar.square(sq[HB:BL, :], ps_b[HB:BL, :CI])
                q = wpool.tile([BL, C], f32, tag="q")
                nc.vector.reduce_sum(
                    q[:], sq[:].rearrange("p (c i) -> p c i", i=DI),
                    axis=mybir.AxisListType.X)
                rt = wpool.tile([BL, C], f32, tag="rt")
                nc.scalar.sqrt(rt[:], q[:])
                den = wpool.tile([BL, C], f32, tag="den")
                nc.scalar.activation(den[:], q[:],
                                     mybir.ActivationFunctionType.Identity,
                                     bias=1.0, scale=a2)
                dr = wpool.tile([BL, C], f32, tag="dr")
                nc.vector.reciprocal(dr[:], den[:])
                g = wpool.tile([BL, C], f32, tag="g")
                nc.vector.tensor_tensor(g[:], rt[:], dr[:],
                                        op=mybir.AluOpType.mult)
                # v = (s*a2) * g  (g broadcast over i)
                nc.vector.scalar_tensor_tensor(
                    v_sb[:HB, :CI].rearrange("p (c i) -> p c i", i=DI),
                    ps_a[:HB, :CI].rearrange("p (c i) -> p c i", i=DI), a2,
                    g[:HB, :].to_broadcast((HB, C, DI)),
                    op0=mybir.AluOpType.mult, op1=mybir.AluOpType.mult)
                nc.vector.scalar_tensor_tensor(
                    v_sb[HB:BL, :CI].rearrange("p (c i) -> p c i", i=DI),
                    ps_b[HB:BL, :CI].rearrange("p (c i) -> p c i", i=DI), a2,
                    g[HB:BL, :].to_broadcast((HB, C, DI)),
                    op0=mybir.AluOpType.mult, op1=mybir.AluOpType.mult)
                if M_TILE:
                    # duplicate v into partitions 64..127 via PE + ACT
                    # (cross-partition move; DMA SBUF->SBUF crashed HW)
                    ps_vd = psum_pool.tile([128, 8 * 256], f32, tag="ps",
                                           name="ps_vd")[:, :CI]
                    nc.tensor.matmul(ps_vd[:], dup_sb[:], v_sb[:BL, :CI],
                                     start=True, stop=True)
                    nc.scalar.copy(v_sb[BL:2 * BL, :CI],
                                   ps_vd[BL:2 * BL, :])

            def agreement(k2_lo, k2_hi):
                """a_loc[n%128, (k2,c)] = sum_{j,i} w * (uf^T @ v)."""
                for k2 in range(k2_lo, k2_hi):
                    pm = psum_pool.tile([128, 8 * 256], f32, tag="ps",
                                        name="ps_m").rearrange(
                        "p (j x) -> p j x", x=256)
                    if M_TILE:
                        for jj in range(DJ // 2):
                            nc.tensor.matmul(
                                pm[:, 2 * jj, :MW],
                                mm_cast(uf[jj][:BL,
                                               k2 * 128:(k2 + 1) * 128]),
                                mm_cast(v_sb[:BL, :]),
                                start=True, stop=True, tile_position=(0, 0))
                            nc.tensor.matmul(
                                pm[:, 2 * jj + 1, :MW],
                                mm_cast(uf[jj][BL:2 * BL,
                                               k2 * 128:(k2 + 1) * 128]),
                                mm_cast(v_sb[BL:2 * BL, :]),
                                start=True, stop=True, tile_position=(BL, 0))
                    elif MC_TILE:
                        # split the 128-wide n-chunk across PE column
                        # groups: two concurrent 64-col matmuls
                        for j in range(DJ):
                            nc.tensor.matmul(
                                pm[:BL, j, :MW],
                                mm_cast(uf[j][:, k2 * 128:k2 * 128 + BL]),
                                mm_cast(v_sb[:BL, :]),
                                start=True, stop=True, tile_position=(0, 0))
                            nc.tensor.matmul(
                                pm[BL:128, j, :MW],
                                mm_cast(uf[j][:, k2 * 128 + BL:
                                              (k2 + 1) * 128]),
                                mm_cast(v_sb[:BL, :]),
                                start=True, stop=True, tile_position=(0, BL))
                    else:
                        for j in range(DJ):
                            nc.tensor.matmul(
                                pm[:, j, :MW],
                                mm_cast(uf[j][:, k2 * 128:(k2 + 1) * 128]),
                                mm_cast(v_sb[:BL, :]),
                                start=True, stop=True)
                    prod = wpool.tile([128, DJ * CI], f32, tag="prod")
                    wslice = w_sb[:].rearrange("p (j k x) -> p j k x",
                                               j=DJ, x=CI)[:, :, k2, :]
                    if G_SPLIT and k2 % 2 == 1:
                        pmev = wpool.tile([128, DJ * CI], f32, tag="pmev")
                        nc.scalar.copy(
                            pmev[:].rearrange("p (j x) -> p j x", x=CI),
                            pm[:, :, :CI])
                        nc.gpsimd.tensor_tensor(
                            prod[:].rearrange("p (j x) -> p j x", x=CI),
                            wslice,
                            pmev[:].rearrange("p (j x) -> p j x", x=CI),
                            op=mybir.AluOpType.mult)
                    else:
                        nc.vector.tensor_tensor(
                            prod[:].rearrange("p (j x) -> p j x", x=CI),
                            wslice,
                            pm[:, :, :CI], op=mybir.AluOpType.mult)
                    # fused sum over (j, i): view [p, c, j, i], reduce XY
                    nc.vector.reduce_sum(
                        a_loc[:, k2 * C:(k2 + 1) * C],
                        prod[:].rearrange("p (j c i) -> p c j i",
                                          j=DJ, c=C, i=DI),
                        axis=mybir.AxisListType.XY)

            def allgather(it, half, k2_lo, k2_hi):
                """AllGather a_loc[:, 10*k2_lo:10*k2_hi] into ag_sb cols."""
                w0, w1 = k2_lo * C, k2_hi * C
                nw = w1 - w0
                cc_in = dpool.tile([128, nw], f32, name=f"cc_in{it}{half}",
                                   tag=f"cc_in{it}{half}")
                cc_out = dpool.tile([N_CORES, 128, nw], f32,
                                    addr_space="Shared",
                                    name=f"cc_out{it}{half}",
                                    tag=f"cc_out{it}{half}")
                nc.sync.dma_start(cc_in[:], a_loc[:, w0:w1])
                if collectives:
                    nc.gpsimd.collective_compute(
                        "AllGather", mybir.AluOpType.bypass,
                        replica_groups=[list(range(N_CORES))],
                        ins=[cc_in.opt()], outs=[cc_out.opt()])
                    nc.sync.dma_start(
                        ag_sb[:, w0 * N_CORES:w1 * N_CORES].rearrange(
                            "p (r x) -> p r x", r=N_CORES),
                        cc_out[:].transpose([1, 0, 2]))
                else:  # cost-sim stand-in for the gather
                    nc.sync.dma_start(ag_sb[:, w0 * N_CORES:w0 * N_CORES + nw],
                                      cc_in[:])

            def update_b(it, half, k2_lo, k2_hi, first):
                w0, w1 = k2_lo * C, k2_hi * C
                agv = ag_sb[:, w0 * N_CORES:w1 * N_CORES].rearrange(
                    "p (r x) -> p x r", r=N_CORES)
                if first:
                    nc.vector.reduce_sum(b_acc[:, w0:w1], agv,
                                         axis=mybir.AxisListType.X)
                else:
                    asum = wpool.tile([128, NB * C], f32, tag="asum")
                    nc.vector.reduce_sum(asum[:, w0:w1], agv,
                                         axis=mybir.AxisListType.X)
                    nc.vector.tensor_tensor(
                        b_acc[:, w0:w1], b_acc[:, w0:w1], asum[:, w0:w1],
                        op=mybir.AluOpType.add)

            KSPLIT = 5

            def agree_and_gather(it, first):
                agreement(0, KSPLIT)
                allgather(it, 0, 0, KSPLIT)  # fires under 2nd-half compute
                agreement(KSPLIT, NB)
                allgather(it, 1, KSPLIT, NB)
                # half-A softmax/cw depend only on AG-A -> they (and the
                # k2<KSPLIT chunks of the next s-matmul) hide AG-B latency
                update_b(it, 0, 0, KSPLIT, first)
                softmax(0, KSPLIT)
                compute_cw(0, KSPLIT)
                update_b(it, 1, KSPLIT, NB, first)
                softmax(KSPLIT, NB)
                compute_cw(KSPLIT, NB)

            # ================= routing =================
            # iter 1: uniform c -> use w directly, fold 0.1 into squash
            ps = s_matmul(rhs=w_sb, rw=CI, k2_outer=False)
            squash(ps, 0.1)
            agree_and_gather(0, first=True)

            ps = s_matmul()
            squash(ps)
            agree_and_gather(1, first=False)

            ps = s_matmul()
            squash(ps)
            nc.sync.dma_start(v_d.ap(), v_sb[:BL, :CI])

    nc.compile()
    return nc


def get_nc(mm_mode=MM_MODE, collectives=True):
    key = (mm_mode, collectives, S_TILE, M_TILE, G_SPLIT, MC_TILE)
    if key not in _CACHE:
        _CACHE[key] = _build(mm_mode, collectives)
    return _CACHE[key]


def make_in_maps(u, W):
    """Host-side layout prep. u [512,1152,8] f32, W [1152,10,16,8] f32."""
    u = np.ascontiguousarray(u, dtype=np.float32)
    W = np.ascontiguousarray(W, dtype=np.float32)
    wj = W.transpose(3, 0, 1, 2).reshape(DJ, NB, 128, CI).transpose(0, 2, 1, 3)
    wj = np.ascontiguousarray(wj.reshape(DJ, 128, NB * CI))
    in_maps = []
    for core in range(N_CORES):
        ul = u[core * BL:(core + 1) * BL]  # [64, 1152, 8]
        utj = ul.transpose(2, 1, 0).reshape(DJ, NB, 128, BL).transpose(
            0, 2, 1, 3)
        utj = np.ascontiguousarray(utj.reshape(DJ, 128, NB * BL))
        ufj = np.ascontiguousarray(ul.transpose(2, 0, 1))  # [8, 64, 1152]
        ufp = ufj.reshape(DJ // 2, 2 * BL, N)  # pair (2jj, 2jj+1) stacked
        m = {}
        for j in range(DJ):
            m[f"ut{j}"] = utj[j]
            m[f"w{j}"] = wj[j]
        if M_TILE:
            for jj in range(DJ // 2):
                m[f"ufp{jj}"] = np.ascontiguousarray(ufp[jj])
            m["dupm"] = np.ascontiguousarray(
                np.tile(np.eye(BL, dtype=np.float32), (1, 2)))
        else:
            for j in range(DJ):
                m[f"uf{j}"] = ufj[j]
        in_maps.append(m)
    return in_maps


def kernel(u, W, _trace=False, _mm_mode=MM_MODE):
    from concourse import bass_utils

    nc = get_nc(_mm_mode)
    in_maps = make_in_maps(u, W)
    res = bass_utils.run_bass_kernel_spmd(
        nc, in_maps, core_ids=list(range(N_CORES)), trace=_trace)
    out = np.empty((B, C, DI), dtype=np.float32)
    for core in range(N_CORES):
        out[core * BL:(core + 1) * BL] = res.results[core]["v"].reshape(
            BL, C, DI)
    if _trace:
        kernel.last_results = res
    return out


# revision 22
# speedup vs baseline: 1.2276x; 1.0769x over previous
"""DigitCaps dynamic-routing kernel for 8 Trainium2 NeuronCores.

Math (reference):
    u: [512, 1152, 8]  W: [1152, 10, 16, 8]
    u_hat[b,n,c,i] = sum_j W[n,c,i,j] u[b,n,j]        (never materialized)
    b=0; for 3 iters: c=softmax(b,axis=1); s=einsum('bnci,nc->bci',u_hat,c);
                      v=squash(s); b+=einsum('bnci,bci->nc',u_hat,v)

Strategy: data-parallel over batch (64 per core). u_hat is recomputed
implicitly inside two factored matmuls per routing iteration:
  s[b,(c,i)]     = u_flat[b,(j,n)] @ (c*W)[(j,n),(c,i)]    (contract 9216)
  M[(j,n),(c,i)] = u_flat^T @ v_flat                       (contract 64)
  agreement a[n,c] = sum_{j,i} W[(j,n),(c,i)] * M[(j,n),(c,i)]   (DVE)
b is shared across batch -> per-core partial agreements are AllGathered
(cheaper than AllReduce at this size) and summed locally each iteration;
the gather is split into two k2-halves so the first AllGather hides under
the second half's compute. The 3rd iteration's agreement is dead code
(b never read again) -> skipped. Iteration 1's uniform c=0.1 uses W
directly with 0.1 folded into squash. Both matmul families run as pairs
of concurrent PE column-tile chains (tile_position col groups; row-group
offsets crash this runtime and are disabled). CW and the agreement
W*M multiplies are split DVE/GPSIMD (ACT evacuates PSUM for GPSIMD).

Layouts (J-outer, flat nj = j*1152 + n, chunks k=(j,k2) of 128 rows):
  ut[(j,n)%128, (j? no) ...]: uT tile [128, (j,k2,b)]   s-matmul lhsT slices
  uf  [b, (j,n)] as 8 tiles [64, 1152]                  M-matmul lhsT slices
  w   [128, (j,k2,c,i)] single tile                     agreement/cw source
  cw  [128, (j,k2,ci padded to MW)]                     s-matmul rhs
MM_MODE "f32r" uses the replicated-fp32 PE mode (1 cy/row at out-free>=256,
vs 4 cy/row for exact fp32); hardware numerics differ from sim - verified
end-to-end on HW against the fp32 reference before adoption.
"""

import sys

sys.path.insert(0, "/opt/trn_rl_repo")

import numpy as np

N_CORES = 8
B = 512
BL = B // N_CORES  # 64 batch per core
N = 1152
C = 10
DI = 16  # output capsule dim
DJ = 8  # input capsule dim
CI = C * DI  # 160
NB = 9  # n-blocks of 128
MM_MODE = "fp32"  # "fp32" | "f32r"
import os as _os
S_TILE = _os.environ.get("S_TILE", "1") == "1"  # PE col-tiling of s-matmul
M_TILE = _os.environ.get("M_TILE", "0") == "1"  # row-tiled agreement mms
G_SPLIT = _os.environ.get("G_SPLIT", "1") == "1"  # GPSIMD offload of DVE work
MC_TILE = _os.environ.get("MC_TILE", "1") == "1"  # col-tiled agreement mms

_CACHE = {}


def _build(mm_mode, collectives=True):
    import concourse.bacc as bacc
    import concourse.mybir as mybir
    import concourse.tile as tile

    f32 = mybir.dt.float32
    f32r = mybir.dt.float32r

    nc = bacc.Bacc("TRN2", target_bir_lowering=False, debug=False,
                   num_devices=N_CORES)

    ut_d = [nc.dram_tensor(f"ut{j}", [128, NB * BL], f32, kind="ExternalInput")
            for j in range(DJ)]
    if M_TILE:
        uf_d = [nc.dram_tensor(f"ufp{jj}", [2 * BL, N], f32,
                               kind="ExternalInput") for jj in range(DJ // 2)]
    else:
        uf_d = [nc.dram_tensor(f"uf{j}", [BL, N], f32, kind="ExternalInput")
                for j in range(DJ)]
    w_d = [nc.dram_tensor(f"w{j}", [128, NB * CI], f32, kind="ExternalInput")
           for j in range(DJ)]
    v_d = nc.dram_tensor("v", [BL, CI], f32, kind="ExternalOutput")
    dup_d = (nc.dram_tensor("dupm", [BL, 128], f32, kind="ExternalInput")
             if M_TILE else None)

    def mm_cast(ap):
        return ap.bitcast(f32r) if mm_mode == "f32r" else ap

    MW = 256 if mm_mode == "f32r" else CI  # matmul rhs/out free width

    with tile.TileContext(nc) as tc:
        with (
            tc.tile_pool(name="const", bufs=1) as cpool,
            tc.tile_pool(name="work", bufs=3) as wpool,
            tc.tile_pool(name="psum", bufs=2, space="PSUM") as psum_pool,
            tc.tile_pool(name="dram", bufs=1, space="DRAM") as dpool,
        ):
            # ---- tiles ----
            w_sb = cpool.tile([128, DJ * NB * CI], f32, tag="w_sb")
            cw = cpool.tile([128, DJ * NB * MW], f32, tag="cw")
            ut = cpool.tile([128, DJ * NB * BL], f32, tag="ut")
            for j in range(DJ):
                nc.sync.dma_start(
                    w_sb[:, j * NB * CI:(j + 1) * NB * CI], w_d[j].ap())
                nc.sync.dma_start(
                    ut[:, j * NB * BL:(j + 1) * NB * BL], ut_d[j].ap())
            uf = []
            if M_TILE:
                for jj in range(DJ // 2):
                    t = cpool.tile([2 * BL, N], f32, tag=f"ufp{jj}",
                                   name=f"ufp{jj}s")
                    nc.sync.dma_start(t[:], uf_d[jj].ap())
                    uf.append(t)
            else:
                for j in range(DJ):
                    t = cpool.tile([BL, N], f32, tag=f"uf{j}",
                                   name=f"uf{j}s")
                    nc.sync.dma_start(t[:], uf_d[j].ap())
                    uf.append(t)

            v_sb = cpool.tile([2 * BL, MW], f32, tag="v_sb")
            if M_TILE:
                dup_sb = cpool.tile([BL, 128], f32, tag="dup_sb")
                nc.sync.dma_start(dup_sb[:], dup_d.ap())
            if MW > CI:
                nc.vector.memset(v_sb[:, CI:MW], 0.0)
                nc.vector.memset(cw[:], 0.0)  # pad cols must be 0 once

            ones_c = cpool.tile([BL, C], f32, tag="ones_c")
            nc.vector.memset(ones_c[:], 1.0)
            b_acc = cpool.tile([128, NB * C], f32, tag="b_acc")
            c_sb = cpool.tile([128, NB * C], f32, tag="c_sb")
            a_loc = cpool.tile([128, NB * C], f32, tag="a_loc")
            ag_sb = cpool.tile([128, N_CORES * NB * C], f32, tag="ag_sb")

            def wv(j):  # w view [128, k2, c, i] for one j
                return w_sb[:, j * NB * CI:(j + 1) * NB * CI].rearrange(
                    "p (k c i) -> p k c i", c=C, i=DI)

            def cwv(j):  # cw view [128, k2, c, i] (MW-strided) for one j
                return cw[:, j * NB * MW:(j + 1) * NB * MW].rearrange(
                    "p (k x) -> p k x", x=MW)[:, :, :CI].rearrange(
                    "p k (c i) -> p k c i", i=DI)

            JV = 5 if G_SPLIT else DJ  # j's on DVE; rest on GPSIMD
            KSPLIT = 5  # k2 boundary of the two AllGather halves

            def compute_cw1():
                # iteration-1 c is uniform 0.1: cw = 0.1 * w (single-src, 2x)
                for j in range(DJ):
                    eng = nc.vector if j < JV else nc.gpsimd
                    eng.tensor_scalar_mul(cwv(j), wv(j), 0.1)

            def compute_cw(k2_lo=0, k2_hi=NB):
                nk = k2_hi - k2_lo
                cb = c_sb[:, k2_lo * C:k2_hi * C].rearrange(
                    "p (k c) -> p k c", c=C).to_broadcast((128, nk, C, DI))
                for j in range(DJ):
                    eng = nc.vector if j < JV else nc.gpsimd
                    eng.tensor_tensor(cwv(j)[:, k2_lo:k2_hi, :, :],
                                      wv(j)[:, k2_lo:k2_hi, :, :], cb,
                                      op=mybir.AluOpType.mult)

            def softmax(k2_lo=0, k2_hi=NB):
                # per-n row softmax: columns [k2_lo*C, k2_hi*C) are
                # independent of the rest -> can run per AllGather half
                nk = k2_hi - k2_lo
                w0, w1 = k2_lo * C, k2_hi * C
                b3 = b_acc[:, w0:w1].rearrange("p (k c) -> p k c", c=C)
                m = wpool.tile([128, NB], f32, tag="sm_m", name="sm_m")[:, :nk]
                nc.vector.reduce_max(m, b3, axis=mybir.AxisListType.X)
                d = wpool.tile([128, NB * C], f32, tag="sm_d", name="sm_d")[:, :nk * C]
                nc.vector.tensor_tensor(
                    d.rearrange("p (k c) -> p k c", c=C), b3,
                    m.to_broadcast((128, nk, C)),
                    op=mybir.AluOpType.subtract)
                e = wpool.tile([128, NB * C], f32, tag="sm_e", name="sm_e")[:, :nk * C]
                nc.scalar.activation(e, d,
                                     mybir.ActivationFunctionType.Exp)
                ssum = wpool.tile([128, NB], f32, tag="sm_s", name="sm_s")[:, :nk]
                nc.vector.reduce_sum(
                    ssum, e.rearrange("p (k c) -> p k c", c=C),
                    axis=mybir.AxisListType.X)
                rec = wpool.tile([128, NB], f32, tag="sm_r", name="sm_r")[:, :nk]
                nc.vector.reciprocal(rec, ssum)
                nc.vector.tensor_tensor(
                    c_sb[:, w0:w1].rearrange("p (k c) -> p k c", c=C),
                    e.rearrange("p (k c) -> p k c", c=C),
                    rec.to_broadcast((128, nk, C)), op=mybir.AluOpType.mult)

            def s_matmul(rhs=None, rw=None, k2_outer=True):
                # batch split into two 32-row halves on distinct PE
                # column-groups -> the two accumulation chains run
                # concurrently on the 128x128 array (col tiling).
                rhs = cw if rhs is None else rhs
                rw = MW if rw is None else rw
                HB = BL // 2
                ps_a = psum_pool.tile([BL, 8 * 256], f32, tag="ps",
                                      name="ps_sa")[:, :MW]
                ps_b = psum_pool.tile([BL, 8 * 256], f32, tag="ps",
                                      name="ps_sb")[:, :MW]
                nk = DJ * NB
                if k2_outer:  # half-A chunks first (AG-B still in flight)
                    order = [j * NB + k2 for k2 in range(NB)
                             for j in range(DJ)]
                else:  # j-outer: follows the per-j ut/w DMA arrival order
                    order = list(range(nk))
                if S_TILE:
                    for t, k in enumerate(order):
                        nc.tensor.matmul(
                            ps_a[:HB, :],
                            mm_cast(ut[:, k * BL:k * BL + HB]),
                            mm_cast(rhs[:, k * rw:k * rw + rw]),
                            start=(t == 0), stop=(t == nk - 1),
                            tile_position=(0, 0))
                        nc.tensor.matmul(
                            ps_b[HB:BL, :],
                            mm_cast(ut[:, k * BL + HB:(k + 1) * BL]),
                            mm_cast(rhs[:, k * rw:k * rw + rw]),
                            start=(t == 0), stop=(t == nk - 1),
                            tile_position=(0, HB))
                else:
                    for t, k in enumerate(order):
                        nc.tensor.matmul(
                            ps_a[:BL, :],
                            mm_cast(ut[:, k * BL:(k + 1) * BL]),
                            mm_cast(rhs[:, k * rw:k * rw + rw]),
                            start=(t == 0), stop=(t == nk - 1))
                return (ps_a, ps_b)

            def squash(ps, alpha=1.0):
                # s_true = alpha*ps; v = g*ps with
                # g = alpha^2*sqrt(q)/(1+alpha^2*q), q = sum_i ps^2
                a2 = alpha * alpha
                ps_a, ps_b = ps
                if not S_TILE:
                    ps_b = ps_a
                HB = BL // 2
                sq = wpool.tile([BL, CI], f32, tag="sq")
                nc.scalar.square(sq[:HB, :], ps_a[:HB, :CI])
                nc.scalar.square(sq[HB:BL, :], ps_b[HB:BL, :CI])
                q = wpool.tile([BL, C], f32, tag="q")
                nc.vector.reduce_sum(
                    q[:], sq[:].rearrange("p (c i) -> p c i", i=DI),
                    axis=mybir.AxisListType.X)
                rt = wpool.tile([BL, C], f32, tag="rt")
                nc.scalar.sqrt(rt[:], q[:])
                den = wpool.tile([BL, C], f32, tag="den")
                nc.vector.scalar_tensor_tensor(
                    den[:], q[:], a2, ones_c[:],
                    op0=mybir.AluOpType.mult, op1=mybir.AluOpType.add)
                dr = wpool.tile([BL, C], f32, tag="dr")
                nc.vector.reciprocal(dr[:], den[:])
                g = wpool.tile([BL, C], f32, tag="g")
                nc.vector.tensor_tensor(g[:], rt[:], dr[:],
                                        op=mybir.AluOpType.mult)
                # v = (s*a2) * g  (g broadcast over i)
                nc.vector.scalar_tensor_tensor(
                    v_sb[:HB, :CI].rearrange("p (c i) -> p c i", i=DI),
                    ps_a[:HB, :CI].rearrange("p (c i) -> p c i", i=DI), a2,
                    g[:HB, :].to_broadcast((HB, C, DI)),
                    op0=mybir.AluOpType.mult, op1=mybir.AluOpType.mult)
                nc.vector.scalar_tensor_tensor(
                    v_sb[HB:BL, :CI].rearrange("p (c i) -> p c i", i=DI),
                    ps_b[HB:BL, :CI].rearrange("p (c i) -> p c i", i=DI), a2,
                    g[HB:BL, :].to_broadcast((HB, C, DI)),
                    op0=mybir.AluOpType.mult, op1=mybir.AluOpType.mult)
                if M_TILE:
                    # duplicate v into partitions 64..127 via PE + ACT
                    # (cross-partition move; DMA SBUF->SBUF crashed HW)
                    ps_vd = psum_pool.tile([128, 8 * 256], f32, tag="ps",
                                           name="ps_vd")[:, :CI]
                    nc.tensor.matmul(ps_vd[:], dup_sb[:], v_sb[:BL, :CI],
                                     start=True, stop=True)
                    nc.scalar.copy(v_sb[BL:2 * BL, :CI],
                                   ps_vd[BL:2 * BL, :])

            def agreement(k2_lo, k2_hi):
                """a_loc[n%128, (k2,c)] = sum_{j,i} w * (uf^T @ v)."""
                for k2 in range(k2_lo, k2_hi):
                    pm = psum_pool.tile([128, 8 * 256], f32, tag="ps",
                                        name="ps_m").rearrange(
                        "p (j x) -> p j x", x=256)
                    if M_TILE:
                        for jj in range(DJ // 2):
                            nc.tensor.matmul(
                                pm[:, 2 * jj, :MW],
                                mm_cast(uf[jj][:BL,
                                               k2 * 128:(k2 + 1) * 128]),
                                mm_cast(v_sb[:BL, :]),
                                start=True, stop=True, tile_position=(0, 0))
                            nc.tensor.matmul(
                                pm[:, 2 * jj + 1, :MW],
                                mm_cast(uf[jj][BL:2 * BL,
                                               k2 * 128:(k2 + 1) * 128]),
                                mm_cast(v_sb[BL:2 * BL, :]),
                                start=True, stop=True, tile_position=(BL, 0))
                    elif MC_TILE:
                        # split the 128-wide n-chunk across PE column
                        # groups: two concurrent 64-col matmuls
                        for j in range(DJ):
                            nc.tensor.matmul(
                                pm[:BL, j, :MW],
                                mm_cast(uf[j][:, k2 * 128:k2 * 128 + BL]),
                                mm_cast(v_sb[:BL, :]),
                                start=True, stop=True, tile_position=(0, 0))
                            nc.tensor.matmul(
                                pm[BL:128, j, :MW],
                                mm_cast(uf[j][:, k2 * 128 + BL:
                                              (k2 + 1) * 128]),
                                mm_cast(v_sb[:BL, :]),
                                start=True, stop=True, tile_position=(0, BL))
                    else:
                        for j in range(DJ):
                            nc.tensor.matmul(
                                pm[:, j, :MW],
                                mm_cast(uf[j][:, k2 * 128:(k2 + 1) * 128]),
                                mm_cast(v_sb[:BL, :]),
                                start=True, stop=True)
                    prod = wpool.tile([128, DJ * CI], f32, tag="prod")
                    wslice = w_sb[:].rearrange("p (j k x) -> p j k x",
                                               j=DJ, x=CI)[:, :, k2, :]
                    if G_SPLIT and k2 not in (KSPLIT - 1, NB - 1):
                        pmev = wpool.tile([128, DJ * CI], f32, tag="pmev")
                        nc.scalar.copy(
                            pmev[:].rearrange("p (j x) -> p j x", x=CI),
                            pm[:, :, :CI])
                        nc.gpsimd.tensor_tensor(
                            prod[:].rearrange("p (j x) -> p j x", x=CI),
                            wslice,
                            pmev[:].rearrange("p (j x) -> p j x", x=CI),
                            op=mybir.AluOpType.mult)
                    else:
                        nc.vector.tensor_tensor(
                            prod[:].rearrange("p (j x) -> p j x", x=CI),
                            wslice,
                            pm[:, :, :CI], op=mybir.AluOpType.mult)
                    # fused sum over (j, i): view [p, c, j, i], reduce XY
                    nc.vector.reduce_sum(
                        a_loc[:, k2 * C:(k2 + 1) * C],
                        prod[:].rearrange("p (j c i) -> p c j i",
                                          j=DJ, c=C, i=DI),
                        axis=mybir.AxisListType.XY)

            def allgather(it, half, k2_lo, k2_hi):
                """AllGather a_loc[:, 10*k2_lo:10*k2_hi] into ag_sb cols."""
                w0, w1 = k2_lo * C, k2_hi * C
                nw = w1 - w0
                cc_in = dpool.tile([128, nw], f32, name=f"cc_in{it}{half}",
                                   tag=f"cc_in{it}{half}")
                cc_out = dpool.tile([N_CORES, 128, nw], f32,
                                    addr_space="Shared",
                                    name=f"cc_out{it}{half}",
                                    tag=f"cc_out{it}{half}")
                nc.sync.dma_start(cc_in[:], a_loc[:, w0:w1])
                if collectives:
                    nc.gpsimd.collective_compute(
                        "AllGather", mybir.AluOpType.bypass,
                        replica_groups=[list(range(N_CORES))],
                        ins=[cc_in.opt()], outs=[cc_out.opt()])
                    nc.sync.dma_start(
                        ag_sb[:, w0 * N_CORES:w1 * N_CORES].rearrange(
                            "p (r x) -> p r x", r=N_CORES),
                        cc_out[:].transpose([1, 0, 2]))
                else:  # cost-sim stand-in for the gather
                    nc.sync.dma_start(ag_sb[:, w0 * N_CORES:w0 * N_CORES + nw],
                                      cc_in[:])

            def update_b(it, half, k2_lo, k2_hi, first):
                w0, w1 = k2_lo * C, k2_hi * C
                agv = ag_sb[:, w0 * N_CORES:w1 * N_CORES].rearrange(
                    "p (r x) -> p x r", r=N_CORES)
                if first:
                    nc.vector.reduce_sum(b_acc[:, w0:w1], agv,
                                         axis=mybir.AxisListType.X)
                else:
                    asum = wpool.tile([128, NB * C], f32, tag="asum")
                    nc.vector.reduce_sum(asum[:, w0:w1], agv,
                                         axis=mybir.AxisListType.X)
                    nc.vector.tensor_tensor(
                        b_acc[:, w0:w1], b_acc[:, w0:w1], asum[:, w0:w1],
                        op=mybir.AluOpType.add)

            def agree_and_gather(it, first):
                agreement(0, KSPLIT)
                allgather(it, 0, 0, KSPLIT)  # fires under 2nd-half compute
                agreement(KSPLIT, NB)
                allgather(it, 1, KSPLIT, NB)
                # half-A softmax/cw depend only on AG-A -> they (and the
                # k2<KSPLIT chunks of the next s-matmul) hide AG-B latency
                update_b(it, 0, 0, KSPLIT, first)
                softmax(0, KSPLIT)
                compute_cw(0, KSPLIT)
                update_b(it, 1, KSPLIT, NB, first)
                softmax(KSPLIT, NB)
                compute_cw(KSPLIT, NB)

            # ================= routing =================
            # iter 1: uniform c -> use w directly, fold 0.1 into squash
            ps = s_matmul(rhs=w_sb, rw=CI, k2_outer=False)
            squash(ps, 0.1)
            agree_and_gather(0, first=True)

            ps = s_matmul()
            squash(ps)
            agree_and_gather(1, first=False)

            ps = s_matmul()
            squash(ps)
            nc.sync.dma_start(v_d.ap(), v_sb[:BL, :CI])

    nc.compile()
    return nc


def get_nc(mm_mode=MM_MODE, collectives=True):
    key = (mm_mode, collectives, S_TILE, M_TILE, G_SPLIT, MC_TILE)
    if key not in _CACHE:
        _CACHE[key] = _build(mm_mode, collectives)
    return _CACHE[key]


def make_in_maps(u, W):
    """Host-side layout prep. u [512,1152,8] f32, W [1152,10,16,8] f32."""
    u = np.ascontiguousarray(u, dtype=np.float32)
    W = np.ascontiguousarray(W, dtype=np.float32)
    wj = W.transpose(3, 0, 1, 2).reshape(DJ, NB, 128, CI).transpose(0, 2, 1, 3)
    wj = np.ascontiguousarray(wj.reshape(DJ, 128, NB * CI))
    in_maps = []
    for core in range(N_CORES):
        ul = u[core * BL:(core + 1) * BL]  # [64, 1152, 8]
        utj = ul.transpose(2, 1, 0).reshape(DJ, NB, 128, BL).transpose(
            0, 2, 1, 3)
        utj = np.ascontiguousarray(utj.reshape(DJ, 128, NB * BL))
        ufj = np.ascontiguousarray(ul.transpose(2, 0, 1))  # [8, 64, 1152]
        ufp = ufj.reshape(DJ // 2, 2 * BL, N)  # pair (2jj, 2jj+1) stacked
        m = {}
        for j in range(DJ):
            m[f"ut{j}"] = utj[j]
            m[f"w{j}"] = wj[j]
        if M_TILE:
            for jj in range(DJ // 2):
                m[f"ufp{jj}"] = np.ascontiguousarray(ufp[jj])
            m["dupm"] = np.ascontiguousarray(
                np.tile(np.eye(BL, dtype=np.float32), (1, 2)))
        else:
            for j in range(DJ):
                m[f"uf{j}"] = ufj[j]
        in_maps.append(m)
    return in_maps


def kernel(u, W, _trace=False, _mm_mode=MM_MODE):
    from concourse import bass_utils

    nc = get_nc(_mm_mode)
    in_maps = make_in_maps(u, W)
    res = bass_utils.run_bass_kernel_spmd(
        nc, in_maps, core_ids=list(range(N_CORES)), trace=_trace)
    out = np.empty((B, C, DI), dtype=np.float32)
    for core in range(N_CORES):
        out[core * BL:(core + 1) * BL] = res.results[core]["v"].reshape(
            BL, C, DI)
    if _trace:
        kernel.last_results = res
    return out
